# revision 1
# baseline (speedup 1.0000x reference)
"""GCN classifier on 8 TRN2 NeuronCores.

Row-shards the 16384-node graph across 8 cores (2048 rows each). All
activations stay feature-major ([feat, nodes]) on chip so every matmul
contracts over the partition dimension. Pass 1 streams the 1 GiB fp32
adjacency once with 1-MiB DMAs split across both HWDGE rings: casts to bf16,
transposes 128x128 blocks on the TensorEngine, computes row degrees with
ones-matmuls over the transposed tiles, and writes A^T to DRAM as fp8-e4m3
in 8-MiB batched SWDGE writes. The two GCN aggregation passes then stream
A^T back (4 j-blocks per DMA, alternating SP/ACT rings) into fp32-PSUM
matmuls against the AllGathered bf16 scaled features; dis-scaling, bias and
relu run fused on DVE/ACT out of PSUM. deg/dis never needs a collective; the
two AllGathers move ~1 MiB/rank each.

Self-contained: hardcodes shapes for nn_GCNClassifer_6786048328674
(relative error vs the fp64 reference ~7e-4, gate 2e-2).
"""

import sys

sys.path.insert(0, "/opt/trn_rl_repo")

from contextlib import ExitStack

import numpy as np

import concourse.bass as bass
from concourse import bacc
import concourse.mybir as mybir
from concourse.tile import TileContext, add_dep_helper
from concourse.bass_utils import run_bass_kernel_spmd
from concourse.masks import make_identity

F32 = mybir.dt.float32
BF16 = mybir.dt.bfloat16
AF = mybir.ActivationFunctionType
ALU = mybir.AluOpType

BN_EPS = 1e-5
N_CORES = 8
P = 128
A_DT = "fp8"           # "bf16" | "fp8" storage dtype for the cached A^T
STOP_AFTER_PASS1 = False


def build_nc(N=16384, F=1024, D1=512, E=256, H=256, G=128, C=10, n_cores=N_CORES):
    R = N // n_cores            # rows (nodes) per core
    assert R % 512 == 0 and N % 2048 == 0
    IB = R // P                 # 128-row blocks per core
    JW = 512                    # pass-1 column chunk width
    JC = N // JW                # pass-1 column chunks
    JO = N // P                 # 128-wide j blocks (passes 2/3)
    JB = 4                      # j-blocks batched per DMA in passes 2/3
    IC = R // 512               # 512-wide i chunks
    ADT = BF16 if A_DT == "bf16" else mybir.dt.float8e4

    nc = bacc.Bacc(num_devices=n_cores)

    # ---- I/O -------------------------------------------------------------
    a_d = nc.declare_dram_parameter("a", [R, N], F32, isOutput=False)
    xT_d = nc.declare_dram_parameter("xT", [F, R], F32, isOutput=False)
    w1_d = nc.declare_dram_parameter("w1", [F, D1], F32, isOutput=False)
    k1_d = nc.declare_dram_parameter("k1", [D1], F32, isOutput=False)
    c1_d = nc.declare_dram_parameter("c1", [D1], F32, isOutput=False)
    w2_d = nc.declare_dram_parameter("w2", [D1, E], F32, isOutput=False)
    k2_d = nc.declare_dram_parameter("k2", [E], F32, isOutput=False)
    c2_d = nc.declare_dram_parameter("c2", [E], F32, isOutput=False)
    g1w_d = nc.declare_dram_parameter("g1w", [E, H], F32, isOutput=False)
    g1b_d = nc.declare_dram_parameter("g1b", [H], F32, isOutput=False)
    g2w_d = nc.declare_dram_parameter("g2w", [H, G], F32, isOutput=False)
    g2b_d = nc.declare_dram_parameter("g2b", [G], F32, isOutput=False)
    cw_d = nc.declare_dram_parameter("cw", [G, C], F32, isOutput=False)
    cb_d = nc.declare_dram_parameter("cb", [C], F32, isOutput=False)
    out_d = nc.declare_dram_parameter("out", [C, R], F32, isOutput=True)

    # ---- collective DRAM tensors ----------------------------------------
    s1_loc = nc.dram_tensor("s1_loc", [R, E], BF16)
    S1g = nc.dram_tensor("S1g", [N, E], BF16, addr_space="Shared")
    s2_loc = nc.dram_tensor("s2_loc", [R, G], BF16)
    S2g = nc.dram_tensor("S2g", [N, G], BF16, addr_space="Shared")
    groups = [list(range(n_cores))]

    with TileContext(nc) as tc, ExitStack() as ctx:
        wpool = ctx.enter_context(tc.tile_pool(name="wpool", bufs=1))
        dram = ctx.enter_context(tc.tile_pool(name="dram", bufs=1, space="DRAM"))
        io_pool = ctx.enter_context(tc.tile_pool(name="io", bufs=3))
        io2_pool = ctx.enter_context(tc.tile_pool(name="io2", bufs=2))
        xio_pool = ctx.enter_context(tc.tile_pool(name="xio", bufs=1))
        psum = ctx.enter_context(tc.tile_pool(name="psum", bufs=1, space="PSUM"))
        _psn = [0]

        def ps_tile(shape, dtype, banks=range(8), name=None):
            tag = "b%d" % (list(banks)[_psn[0] % len(list(banks))])
            _psn[0] += 1
            return psum.tile(shape, dtype, tag=tag, name=name or f"ps{_psn[0]}")

        # ---- constants / weights in SBUF --------------------------------
        idb = wpool.tile([P, P], BF16)
        make_identity(nc, idb)
        ones_b = wpool.tile([P, 1], BF16)
        nc.vector.memset(ones_b, 1.0)

        w1_sb = wpool.tile([P, F // P, D1], BF16)
        nc.gpsimd.dma_start(w1_sb, w1_d.ap().rearrange("(ko p) m -> p ko m", p=P))
        w2_sb = wpool.tile([P, D1 // P, E], BF16)
        nc.gpsimd.dma_start(w2_sb, w2_d.ap().rearrange("(ko p) m -> p ko m", p=P))
        g1w_sb = wpool.tile([P, E // P, H], F32)
        nc.sync.dma_start(g1w_sb, g1w_d.ap().rearrange("(ko p) m -> p ko m", p=P))
        g2w_sb = wpool.tile([P, H // P, G], F32)
        nc.sync.dma_start(g2w_sb, g2w_d.ap().rearrange("(ko p) m -> p ko m", p=P))
        cw_sb = wpool.tile([G, C], F32)
        nc.sync.dma_start(cw_sb, cw_d[:, :])

        def load_vec(d, n, nm):
            t = wpool.tile([P, n // P], F32, tag=nm, name=nm)
            nc.sync.dma_start(t, d.ap().rearrange("(o p) -> p o", p=P))
            return t

        k1_sb = load_vec(k1_d, D1, "k1v")
        c1_sb = load_vec(c1_d, D1, "c1v")
        k2_sb = load_vec(k2_d, E, "k2v")
        c2_sb = load_vec(c2_d, E, "c2v")
        g1b_sb = load_vec(g1b_d, H, "g1bv")
        g2b_sb = load_vec(g2b_d, G, "g2bv")
        cb_sb = wpool.tile([C, 1], F32)
        nc.sync.dma_start(cb_sb, cb_d.ap().rearrange("(c o) -> c o", o=1))

        # ---- persistent activations (feature-major) ----------------------
        h1_sb = wpool.tile([P, D1 // P, R], BF16, tag="tagA")  # 2 MiB
        h2_sb = wpool.tile([P, E // P, R], F32, tag="tagB")    # 2 MiB
        xw1_sb = wpool.tile([P, H // P, R], F32, tag="tagC")   # 2 MiB
        s1T_sb = wpool.tile([P, H // P, R], BF16)              # 1 MiB
        s2T_sb = wpool.tile([P, R], BF16)                      # 0.5 MiB
        out_sb = wpool.tile([C, R], F32)
        dis_bc = wpool.tile([P, R], F32)                       # dis broadcast, 1 MiB

        # =========== encoder: h1 = relu(bn1(X@W1+b1)), h2, xw1 ===========
        xT_r = xT_d.ap().rearrange("(ko p) i -> p ko i", p=P)
        for s in range(IC):
            isl = bass.ts(s, 512)
            xs = xio_pool.tile([P, F // P, 512], BF16, tag="xstrip")
            nc.gpsimd.dma_start(xs, xT_r[:, :, isl])
            for m in range(D1 // P):
                ps = ps_tile([P, 512], F32, banks=range(4))
                for k in range(F // P):
                    nc.tensor.matmul(ps, w1_sb[:, k, bass.ts(m, P)], xs[:, k],
                                     start=(k == 0), stop=(k == F // P - 1))
                nc.scalar.activation(h1_sb[:, m, isl], ps, AF.Relu,
                                     bias=c1_sb[:, m:m + 1], scale=k1_sb[:, m:m + 1])
            for m in range(E // P):
                ps = ps_tile([P, 512], F32, banks=range(4))
                for k in range(D1 // P):
                    nc.tensor.matmul(ps, w2_sb[:, k, bass.ts(m, P)], h1_sb[:, k, isl],
                                     start=(k == 0), stop=(k == D1 // P - 1))
                nc.scalar.activation(h2_sb[:, m, isl], ps, AF.Relu,
                                     bias=c2_sb[:, m:m + 1], scale=k2_sb[:, m:m + 1])
            for m in range(H // P):
                ps = ps_tile([P, 512], F32, banks=range(4))
                for k in range(E // P):
                    nc.tensor.matmul(ps, g1w_sb[:, k, bass.ts(m, P)], h2_sb[:, k, isl],
                                     start=(k == 0), stop=(k == E // P - 1))
                nc.vector.tensor_copy(xw1_sb[:, m, isl], ps)

        # =========== pass 1: stream A, cast, PE-transpose, col-degs =======
        a_q = dram.tile([N, R], ADT)
        a_q_w = a_q.rearrange("(g t p) i -> p (g t) i", p=P, t=JW // P)
        dps_row = [psum.tile([1, 512], F32, tag=f"b{4 + i}", name=f"degps{i}")
                   for i in range(IC)]
        JG = 2048                     # columns per group
        NT = JG // P                  # 16 transposed blocks per group
        for jg in range(N // JG):
            wide = io2_pool.tile([P, NT, R], ADT, tag="wide")
            for ib in range(IB):
                at = io2_pool.tile([P, JG], F32, tag="a_in")
                eng_d = nc.sync if ib % 2 == 0 else nc.scalar
                eng_d.dma_start(at, a_d[bass.ts(ib, P), bass.ts(jg, JG)])
                ab = io2_pool.tile([P, JG], BF16, tag="a_cast")
                if ib % 2 == 0:
                    nc.scalar.activation(ab, at, AF.Copy)
                else:
                    nc.vector.tensor_copy(ab, at)
                for half in range(2):
                    pst = ps_tile([P, JG // 2], BF16, banks=range(4))
                    for t in range(NT // 2):
                        tt_ = half * (NT // 2) + t
                        nc.tensor.transpose(pst[:, bass.ts(t, P)],
                                            ab[:, bass.ts(tt_, P)], idb)
                    dst = wide[:, half * (NT // 2):(half + 1) * (NT // 2),
                               bass.ts(ib, P)]
                    if (ib + half) % 2 == 0:
                        nc.vector.tensor_copy(
                            dst, pst.rearrange("p (t i) -> p t i", t=NT // 2))
                    else:
                        nc.scalar.activation(
                            dst, pst.rearrange("p (t i) -> p t i", t=NT // 2),
                            AF.Copy)
            for t in range(NT):
                for i in range(IC):
                    nc.tensor.matmul(
                        dps_row[i], ones_b, wide[:, t, bass.ts(i, 512)],
                        start=(jg == 0 and t == 0),
                        stop=(jg == N // JG - 1 and t == NT - 1))
            nc.gpsimd.dma_start(a_q_w[:, bass.ts(jg, NT), :], wide)

        # own-row degrees -> dis (no collective needed)
        dmy = wpool.tile([1, R], F32)
        for i in range(IC):
            nc.vector.tensor_copy(dmy[:, bass.ts(i, 512)], dps_row[i])
        nc.vector.reciprocal(dmy, dmy)
        nc.scalar.activation(dmy, dmy, AF.Sqrt)
        dis_dram = dram.tile([1, R], F32)
        nc.sync.dma_start(dis_dram, dmy)
        nc.sync.dma_start(dis_bc, dis_dram[0:1, :].to_broadcast([P, R]))

        if STOP_AFTER_PASS1:
            nc.vector.tensor_copy(out_sb, dis_bc[0:C, :])
            nc.sync.dma_start(out_d[:, :], out_sb)
        else:
            # ======= s1 = dis * xw1 -> natural layout -> AllGather ========
            for m in range(H // P):
                nc.vector.tensor_tensor(s1T_sb[:, m], xw1_sb[:, m], dis_bc,
                                        ALU.mult)
            for it in range(IB):
                pst = ps_tile([P, H], BF16, banks=range(4))
                for m in range(H // P):
                    nc.tensor.transpose(pst[:, bass.ts(m, P)],
                                        s1T_sb[:, m, bass.ts(it, P)], idb)
                snat = io_pool.tile([P, H], BF16, tag="s1nat")
                nc.scalar.activation(snat, pst, AF.Copy)
                nc.gpsimd.dma_start(s1_loc[bass.ts(it, P), :], snat)
            cc1 = nc.gpsimd.collective_compute(
                "AllGather", ALU.bypass, replica_groups=groups,
                ins=[s1_loc[:, :].opt()], outs=[S1g[:, :].opt()])

            # ======= pass 2: y1 = (A @ S1)^T ; h3 = relu(dis*y1 + b) ======
            NB1 = E // P
            a_q_r = a_q.rearrange("(o p) i -> p o i", p=P)
            S1_r = S1g.ap().rearrange("(o p) n -> p o n", p=P)
            S2_r = S2g.ap().rearrange("(o p) n -> p o n", p=P)
            ps_y = [psum.tile([P, 512], F32, tag=f"b{m * IC + i}",
                              name=f"ps_y_{m}_{i}")
                    for m in range(NB1) for i in range(IC)]
            JA = 2 * JB
            for jp in range(JO // JA):
                att = io2_pool.tile([P, JA, R], ADT, tag="wide")
                eng = nc.sync if jp % 2 == 0 else nc.scalar
                eng2 = nc.scalar if jp % 2 == 0 else nc.sync
                eng.dma_start(att, a_q_r[:, jp * JA:(jp + 1) * JA, :])
                for h in range(2):
                    s1t = io_pool.tile([P, JB, E], BF16, tag="sjo")
                    jb0 = jp * JA + h * JB
                    d2 = eng2.dma_start(s1t, S1_r[:, jb0:jb0 + JB, :])
                    add_dep_helper(d2.ins, cc1.ins, reason="S1 read after AG")
                    for q in range(JB):
                        jo = jb0 + q
                        for m in range(NB1):
                            for i in range(IC):
                                nc.tensor.matmul(
                                    ps_y[m * IC + i], s1t[:, q, bass.ts(m, P)],
                                    att[:, h * JB + q, bass.ts(i, 512)],
                                    start=(jo == 0), stop=(jo == JO - 1))
            h3_sb = wpool.tile([P, H // P, R], F32, tag="tagB")
            for m in range(NB1):
                for i in range(IC):
                    isl = bass.ts(i, 512)
                    tt = io_pool.tile([P, 512], F32, tag="ep")
                    nc.vector.tensor_tensor(tt, ps_y[m * IC + i], dis_bc[:, isl],
                                            ALU.mult)
                    nc.scalar.activation(h3_sb[:, m, isl], tt, AF.Relu,
                                         bias=g1b_sb[:, m:m + 1])

            # ======= xw2, s2 = dis*xw2 -> natural -> AllGather ============
            for i in range(IC):
                isl = bass.ts(i, 512)
                ps = ps_tile([P, 512], F32, banks=range(0, 4))
                for k in range(H // P):
                    nc.tensor.matmul(ps, g2w_sb[:, k, :], h3_sb[:, k, isl],
                                     start=(k == 0), stop=(k == H // P - 1))
                nc.vector.tensor_tensor(s2T_sb[:, isl], ps, dis_bc[:, isl],
                                        ALU.mult)
            for it in range(IB):
                pst = ps_tile([P, G], BF16, banks=range(0, 4))
                nc.tensor.transpose(pst, s2T_sb[:, bass.ts(it, P)], idb)
                snat = io_pool.tile([P, G], BF16, tag="s2nat")
                nc.scalar.activation(snat, pst, AF.Copy)
                nc.gpsimd.dma_start(s2_loc[bass.ts(it, P), :], snat)
            cc2 = nc.gpsimd.collective_compute(
                "AllGather", ALU.bypass, replica_groups=groups,
                ins=[s2_loc[:, :].opt()], outs=[S2g[:, :].opt()])

            # ======= pass 3: y2 = (A @ S2)^T ; h4 = relu(dis*y2 + b) ======
            ps_z = [psum.tile([P, 512], F32, tag=f"b{4 + i}", name=f"ps_z_{i}")
                    for i in range(IC)]
            for jp in range(JO // JA):
                att = io2_pool.tile([P, JA, R], ADT, tag="wide")
                eng = nc.sync if jp % 2 == 0 else nc.scalar
                eng2 = nc.scalar if jp % 2 == 0 else nc.sync
                eng.dma_start(att, a_q_r[:, jp * JA:(jp + 1) * JA, :])
                for h in range(2):
                    s2t = io_pool.tile([P, JB, G], BF16, tag="sjo")
                    jb0 = jp * JA + h * JB
                    d2 = eng2.dma_start(s2t, S2_r[:, jb0:jb0 + JB, :])
                    add_dep_helper(d2.ins, cc2.ins, reason="S2 read after AG")
                    for q in range(JB):
                        jo = jb0 + q
                        for i in range(IC):
                            nc.tensor.matmul(ps_z[i], s2t[:, q, :],
                                             att[:, h * JB + q, bass.ts(i, 512)],
                                             start=(jo == 0), stop=(jo == JO - 1))
            h4_sb = wpool.tile([P, R], F32, tag="tagC")
            for i in range(IC):
                isl = bass.ts(i, 512)
                tt = io_pool.tile([P, 512], F32, tag="ep")
                nc.vector.tensor_tensor(tt, ps_z[i], dis_bc[:, isl], ALU.mult)
                nc.scalar.activation(h4_sb[:, isl], tt, AF.Relu,
                                     bias=g2b_sb[:, 0:1])

            # ======= classifier: out = clip(sigmoid(h4 @ cw + cb)) ========
            for i in range(IC):
                isl = bass.ts(i, 512)
                ps = ps_tile([C, 512], F32, banks=range(0, 4))
                nc.tensor.matmul(ps, cw_sb, h4_sb[:, isl], start=True, stop=True)
                nc.scalar.activation(out_sb[:, isl], ps, AF.Sigmoid, bias=cb_sb)
            nc.vector.tensor_scalar(out_sb, out_sb, 1.0 - 1e-10, 1e-10,
                                    ALU.min, ALU.max)
            nc.sync.dma_start(out_d[:, :], out_sb)

    nc.finalize()
    return nc


def make_in_maps(inputs, N, n_cores=N_CORES):
    f = {k: np.ascontiguousarray(np.asarray(v, dtype=np.float32))
         for k, v in inputs.items()}
    k1 = f["bn1_g"] / np.sqrt(f["bn1_v"] + BN_EPS)
    c1 = (f["enc_b1"] - f["bn1_m"]) * k1 + f["bn1_b"]
    k2 = f["bn2_g"] / np.sqrt(f["bn2_v"] + BN_EPS)
    c2 = (f["enc_b2"] - f["bn2_m"]) * k2 + f["bn2_b"]
    R = N // n_cores
    shared = dict(
        w1=f["enc_w1"], k1=k1, c1=c1,
        w2=f["enc_w2"], k2=k2, c2=c2,
        g1w=f["gcn1_w"], g1b=f["gcn1_b"],
        g2w=f["gcn2_w"], g2b=f["gcn2_b"],
        cw=f["cls_w"], cb=f["cls_b"],
    )
    maps = []
    for c in range(n_cores):
        r0, r1 = c * R, (c + 1) * R
        m = dict(shared)
        m["a"] = np.ascontiguousarray(f["adj"][r0:r1])
        m["xT"] = np.ascontiguousarray(f["feature"][r0:r1].T)
        maps.append(m)
    return maps


_NC_CACHE = {}


def run(inputs, trace=False, N=16384, n_cores=N_CORES):
    key = (N, n_cores)
    if key not in _NC_CACHE:
        _NC_CACHE[key] = build_nc(N=N, n_cores=n_cores)
    nc = _NC_CACHE[key]
    in_maps = make_in_maps(inputs, N, n_cores)
    res = run_bass_kernel_spmd(nc, in_maps, core_ids=list(range(n_cores)),
                               trace=trace)
    out = np.concatenate([r["out"].T for r in res.results], axis=0)
    return np.ascontiguousarray(out.astype(np.float32)), res


def kernel(**inputs) -> np.ndarray:
    out, _ = run(inputs, trace=False)
    return out

